# revision 22
# baseline (speedup 1.0000x reference)
"""Multi-head attention (B=2, S=2048, D=1024, H=16, dk=dv=64) on 8 TRN2 cores.

Sharding: core c -> batch b = c % 2, head-group g = c // 2 (heads 4g..4g+3).
Each core computes its 4 heads' attention for one batch plus the partial
output projection; the host sums the 4 partials per batch and adds bo.

Host marshalling: inputs are sliced per batch, transposed to [D, S]
(the PE contracts over the partition dim, so projections need D-major
operands), cast to bf16, and the per-head weights are packed/stacked in
bf16; the reference's softmax/dk/2 scale is folded into Wv and bv.

All matmul operands are bf16 (1 cycle/row, and roughly half the PE power
draw of float32r, which matters: the HW activity monitor duty-cycles the
PE to 4/8 under sustained fp32r load). PSUM accumulation stays fp32.

Engine split: PE does matmuls only; ACT does ONLY the exp evictions (it
is the serial bottleneck of the attention inner loop); DVE does proj
PSUM evictions with fused bias + the softmax normalize; GPSIMD (Pool)
does the out-projection PSUM evictions.

Per-core device pipeline:
  1. QWT/KWT [dk, S] head projections (heads pair-stacked on partitions,
     biases fused into the DVE PSUM->SBUF eviction), VW [S, dv] natural
     (via PE transpose; softmax-denominator ones column DMA'd in).
  2. scoresT[t, s] = KWT.T @ QWT per head, two heads concurrently via
     64x128 PE row tiling; exp fused into the PSUM->SBUF eviction (ACT).
     No max-subtraction (|scores| < 50, exp stays finite in fp32/bf16).
  3. ctxT[dv+1, s] = VW1.T @ exp_scoresT accumulated over t; row dv is the
     softmax denominator. Normalize: one K=2 matmul broadcasts both heads'
     denominator rows to 64 partitions each, DVE reciprocal, DVE multiply.
  4. out[s, D] partial = ctx_allT.T @ Wo_slice, Pool-evicted to bf16,
     DMA'd out (host accumulates partials in fp32).
"""
import os
import sys

sys.path.insert(0, "/opt/trn_rl_repo")
os.environ.setdefault("JAX_PLATFORMS", "axon,cpu")

from contextlib import ExitStack

import ml_dtypes
import numpy as np

import concourse.bacc as bacc
import concourse.tile as tile
from concourse import mybir
from concourse.bass_utils import run_bass_kernel_spmd

FP32 = mybir.dt.float32
BF16 = mybir.dt.bfloat16
BF16NP = ml_dtypes.bfloat16

B, S, D = 2, 2048, 1024
H, DK, DV = 16, 64, 64
N_CORES = 8
HPC = H // (N_CORES // B)  # heads per core = 4
P = 128
SBLK = 512                # s-block (free dim of scores matmuls)
NBLK = S // SBLK          # 4
NTT = S // P              # 16 t-tiles
NDC = D // P              # 8 contraction chunks
NV = HPC * (DV + 1)       # 260
SCALE = 1.0 / (DK * 2.0)  # folded into Wv/bv


def _build_nc():
    nc = bacc.Bacc("TRN2", target_bir_lowering=False, debug=False,
                   num_devices=N_CORES)
    d = {}
    for name, shape, dt_ in [
        ("qt", [D, S], BF16), ("kt", [D, S], BF16), ("vt", [D, S], BF16),
        ("wq", [D, 2 * P], BF16), ("wk", [D, 2 * P], BF16), ("wv", [D, 2 * P], BF16),
        ("bqk", [P, 6], FP32), ("ident", [P, P], BF16),
        ("wo", [HPC * DV, D], BF16), ("ones2", [2, P], BF16),
    ]:
        d[name] = nc.dram_tensor(name, shape, dt_, kind="ExternalInput").ap()
    out_d = nc.dram_tensor("out", [S, D], BF16, kind="ExternalOutput").ap()
    # [D, S] viewed as [p, dc, s] chunks for DMA
    xt_view = {
        n: d[n].rearrange("(dc p) s -> p dc s", p=P)
        for n in ("qt", "kt", "vt")
    }

    with tile.TileContext(nc) as tc, ExitStack() as ctx:
        const = ctx.enter_context(tc.tile_pool(name="const", bufs=1))
        wpool = ctx.enter_context(tc.tile_pool(name="wpool", bufs=1))
        xtp = ctx.enter_context(tc.tile_pool(name="xtp", bufs=2))
        projp = ctx.enter_context(tc.tile_pool(name="projp", bufs=1))
        expp = ctx.enter_context(tc.tile_pool(name="expp", bufs=1))
        ctxp = ctx.enter_context(tc.tile_pool(name="ctxp", bufs=1))
        outp = ctx.enter_context(tc.tile_pool(name="outp", bufs=2))
        smallp = ctx.enter_context(tc.tile_pool(name="smallp", bufs=2))
        psum = ctx.enter_context(tc.tile_pool(name="psum", bufs=1, space="PSUM"))

        # ---- weights + full K upfront (K projection starts the kernel;
        # everything else trickles in behind the K chunk loads). Small
        # constants go first: they are needed early and cost ~no latency.
        wk_sb = wpool.tile([P, NDC, 2 * P], BF16)
        kt_sb = wpool.tile([P, NDC, S], BF16)
        wkv = d["wk"].rearrange("(dc p) m -> p dc m", p=P)
        # first K-proj accumulation group touches wk[dc] + kt0[dc] in dc
        # order: interleave half-sized loads so the PE starts ~5us earlier.
        for h in range(2):
            nc.sync.dma_start(wk_sb[:, h * 4:(h + 1) * 4, :],
                              wkv[:, h * 4:(h + 1) * 4, :])
            nc.sync.dma_start(kt_sb[:, h * 4:(h + 1) * 4, 0:SBLK],
                              xt_view["kt"][:, h * 4:(h + 1) * 4, 0:SBLK])
        bqk = const.tile([P, 6], FP32)
        nc.sync.dma_start(bqk[:], d["bqk"])
        ones2 = const.tile([2, P], BF16)
        nc.sync.dma_start(ones2[:], d["ones2"])
        ident_r = const.tile([P, P], BF16)
        nc.sync.dma_start(ident_r[:], d["ident"])
        for ci in range(1, NBLK):
            nc.sync.dma_start(kt_sb[:, :, ci * SBLK:(ci + 1) * SBLK],
                              xt_view["kt"][:, :, ci * SBLK:(ci + 1) * SBLK])
        wq_sb = wpool.tile([P, NDC, 2 * P], BF16)
        wv_sb = wpool.tile([P, NDC, 2 * P], BF16)
        wo_sb = wpool.tile([P, 2, D], BF16)

        def load_w(sb, name, pat="(dc p) m -> p dc m"):
            nc.sync.dma_start(sb[:], d[name].rearrange(pat, p=P))

        # ---- persistent activation tiles ----
        qwt = [projp.tile([P, S], BF16, tag=f"qwt{p_}", name=f"qwt{p_}") for p_ in range(2)]
        kwt = [projp.tile([P, S], BF16, tag=f"kwt{p_}", name=f"kwt{p_}") for p_ in range(2)]
        vwt = [projp.tile([P, S], BF16, tag=f"vwt{p_}", name=f"vwt{p_}") for p_ in range(2)]
        vw = projp.tile([P, NTT, NV], BF16, tag="vw")
        ctx_t = [ctxp.tile([P, S], BF16, tag=f"ctx{p_}", name=f"ctx{p_}") for p_ in range(2)]

        def load_chunk(name, ci, tag="xtk", bufs=2):
            xt = xtp.tile([P, NDC, SBLK], BF16, tag=tag, name="xt", bufs=bufs)
            nc.sync.dma_start(xt[:], xt_view[name][:, :, ci * SBLK:(ci + 1) * SBLK])
            return xt

        def proj_qk_pair(xt, w_sb, dst, bias_col, ci, pair, tag=None, xsl=None):
            """Project one head-pair of a chunk into dst[pair][:, ci*SBLK:...]."""
            pq = psum.tile([P, SBLK], FP32, tag=tag or ("pj" if pair == 0 else "po"),
                           name="pq")
            for dc in range(NDC):
                rhs = xt[:, dc, :] if xsl is None else xt[:, dc, xsl]
                nc.tensor.matmul(pq[:], lhsT=w_sb[:, dc, pair * P:(pair + 1) * P],
                                 rhs=rhs, start=(dc == 0), stop=(dc == NDC - 1))
            nc.vector.tensor_scalar_add(dst[pair][:, ci * SBLK:(ci + 1) * SBLK], pq[:],
                                        bqk[:, bias_col + pair:bias_col + pair + 1])

        def proj_v_pair(xt, ci, pair, tags=("ct0", "ct1"), ptag=None):
            """VWT head-pair (pair-stacked, like Q/K), then PE-transpose into
            vw natural. `tags` picks the transpose PSUM tags (ct0/ct1 are
            free pre-attention; pj/po when run as an attention filler)."""
            proj_qk_pair(xt, wv_sb, vwt, 4, ci, pair, tag=ptag)
            for c in range(SBLK // P):
                tt = ci * (SBLK // P) + c
                tp = psum.tile([P, P], BF16, tag=tags[c % 2], name="tp")
                nc.tensor.transpose(
                    tp[:], vwt[pair][:, ci * SBLK + c * P:ci * SBLK + (c + 1) * P],
                    ident_r[:])
                nc.vector.tensor_copy(
                    vw[:, tt, :].rearrange("p (h v) -> p h v", v=DV + 1)[:, 2 * pair:2 * pair + 2, 0:DV],
                    tp[:].rearrange("p (h v) -> p h v", h=2))

        def attn_alloc(pair):
            return [psum.tile([DV + 1, SBLK], FP32, tag=f"ct{hp}", name=f"ct{hp}")
                    for hp in range(2)]

        def attn_block(pair, b, ct, fillers):
            """Per-2-t-tile pipeline: scores(k) -> exp(k) -> ctx(k), ctx chasing
            exp by one step. One 4-bank scores PSUM per step holds both heads'
            2 t-tiles, evicted by a single FD=2048 exp. `fillers` is a list of
            no-arg callables emitting extra PE work, drained one per step."""
            NK = NTT // 2
            exs = {}
            for k in range(NK + 2):
                if k < NK:
                    sc = [psum.tile([P, 2 * SBLK], FP32, tag=f"sc{hp}", name=f"sc{hp}")
                          for hp in range(2)]
                    for sub in range(2):
                        tt = k * 2 + sub
                        for hp in range(2):
                            lo, hi = hp * DK, (hp + 1) * DK
                            nc.tensor.matmul(
                                sc[hp][:, sub * SBLK:(sub + 1) * SBLK],
                                lhsT=kwt[pair][lo:hi, tt * P:(tt + 1) * P],
                                rhs=qwt[pair][lo:hi, b * SBLK:(b + 1) * SBLK],
                                start=True, stop=True)
                    ex = [expp.tile([P, 2, SBLK], BF16, tag=f"exp{hp}", name=f"exp{hp}", bufs=3)
                          for hp in range(2)]
                    for hp in range(2):
                        nc.scalar.activation(
                            ex[hp][:], sc[hp][:].rearrange("p (u q) -> p u q", u=2),
                            mybir.ActivationFunctionType.Exp)
                    exs[k] = ex
                if fillers:
                    fillers.pop(0)()
                # ctx trails exp by 2 steps: the block's first ctx matmul waits
                # for the ct-psum slot freed by the PREVIOUS block's normalize,
                # so give that chain two steps of slack.
                kc = k - 2
                if kc >= 0:
                    ex = exs.pop(kc)
                    for sub in range(2):
                        tt = kc * 2 + sub
                        for hp in range(2):
                            hh = 2 * pair + hp
                            nc.tensor.matmul(
                                ct[hp][:], lhsT=vw[:, tt, hh * (DV + 1):(hh + 1) * (DV + 1)],
                                rhs=ex[hp][:, sub, :],
                                start=(tt == 0), stop=(tt == NTT - 1))

        def attn_normalize(pair, b, ct):
            # ctx = ct[0:64] * (1 / ct[64]) row-broadcast
            for hp in range(2):
                den = smallp.tile([1, SBLK], BF16, tag="den")
                nc.vector.tensor_copy(den[:], ct[hp][DV:DV + 1, :])
                rb = psum.tile([DV, SBLK], FP32, tag="pj", name="rb")
                nc.tensor.matmul(rb[:], lhsT=ones2[0:1, 0:DV], rhs=den[:],
                                 start=True, stop=True)
                rcp = smallp.tile([DV, SBLK], FP32, tag="rcp")
                nc.vector.reciprocal_approx_fast(rcp[:], rb[:])
                nc.vector.tensor_mul(
                    ctx_t[pair][hp * DV:(hp + 1) * DV, b * SBLK:(b + 1) * SBLK],
                    ct[hp][0:DV, :], rcp[:])

        def out_proj_nh(b, st, nh, tag="po", scalar_evict=False):
            off = b * SBLK + st * P
            po = psum.tile([P, SBLK], FP32, tag=tag, name="po")
            for jc in range(2):
                nc.tensor.matmul(po[:],
                                 lhsT=ctx_t[jc][:, off:off + P],
                                 rhs=wo_sb[:, jc, nh * SBLK:(nh + 1) * SBLK],
                                 start=(jc == 0), stop=(jc == 1))
            ob = outp.tile([P, SBLK], BF16, tag="ob", bufs=4)
            if scalar_evict:
                nc.scalar.copy(ob[:], po[:])
            else:
                nc.vector.tensor_copy(ob[:], po[:])
            nc.sync.dma_start(out_d[off:off + P, nh * SBLK:(nh + 1) * SBLK], ob[:])

        def proj_qk_piece(xt, w_sb, dst, bias_col, ci, pair, dc_range, pq_holder):
            if dc_range[0] == 0:
                pq_holder[pair] = psum.tile([P, SBLK], FP32, tag="pj", name="pq")
            pq = pq_holder[pair]
            for dc in dc_range:
                nc.tensor.matmul(pq[:], lhsT=w_sb[:, dc, pair * P:(pair + 1) * P],
                                 rhs=xt[:, dc, :], start=(dc == 0), stop=(dc == NDC - 1))
            if dc_range[-1] == NDC - 1:
                nc.vector.tensor_scalar_add(dst[pair][:, ci * SBLK:(ci + 1) * SBLK], pq[:],
                                            bqk[:, bias_col + pair:bias_col + pair + 1])

        # ---- emission schedule (pair-split early attention) ----
        # Pair-0's K/Q0/V projections run first; attention block (0,0)
        # starts immediately after, with pair-1's K/V/Q0 projections drained
        # into its per-step PE slack. This puts the first exp on ACT (the
        # serial bottleneck) ~25us earlier than projecting everything first.
        qt0 = load_chunk("qt", 0)
        load_w(wq_sb, "wq")
        load_w(wv_sb, "wv")
        vts = {ci: load_chunk("vt", ci, tag="xtv", bufs=4) for ci in range(NBLK)}
        # softmax-denominator ones column (strided over the 65-wide head
        # slots): engine memset, not DMA — an engine op gets proper ordering
        # against the ctx matmuls that read it, and doesn't queue behind
        # ~10MB of input DMA.
        for hh in range(HPC):
            nc.vector.memset(vw[:, :, hh * (DV + 1) + DV], 1.0)
        load_w(wo_sb, "wo", "(jc p) n -> p jc n")

        # PE warmup: the tensor engine p-state ramps with continuous busy
        # time (0.65 -> 1.2 -> 2.4 GHz); the first ~8us are DMA-latency
        # bound anyway, so burn them on dummy matmuls to arrive at the
        # first projection with a hot clock. The dummy PSUM is never read.
        warm = const.tile([P, SBLK], BF16)
        nc.vector.memset(warm[:], 0.0)
        for _ in range(20):
            wp = psum.tile([P, SBLK], FP32, tag="sc0", name="wp")
            nc.tensor.matmul(wp[:], lhsT=warm[:, 0:P], rhs=warm[:],
                             start=True, stop=True)

        # Phase A: K (both pairs, alternating PSUM tags) and Q0 only. All V
        # projections drain into block (0,0)'s filler slots below: ctx step
        # k only needs V chunk k//2, so pair-0 chunk ci is due at step
        # 2ci+2, and pair-1 slots aren't read until block (1,0).
        for ci in range(NBLK):
            for pair in range(2):
                proj_qk_pair(kt_sb, wk_sb, kwt, 2, ci, pair,
                             xsl=slice(ci * SBLK, (ci + 1) * SBLK))
        proj_qk_pair(qt0, wq_sb, qwt, 0, 0, 0)
        proj_qk_pair(qt0, wq_sb, qwt, 0, 0, 1)

        def interleave(a, bl):
            out = []
            for i in range(max(len(a), len(bl))):
                if i < len(a):
                    out.append(a[i])
                if i < len(bl):
                    out.append(bl[i])
            return out

        # Phase B: block (0,0); fillers: all V projections (deadline order).
        qt1 = load_chunk("qt", 1)
        holder1 = [None, None]
        fillB = []
        for ci, pr in [(0, 0), (1, 0), (0, 1), (2, 0), (1, 1), (3, 0),
                       (2, 1), (3, 1)]:
            fillB.append(lambda c=ci, p=pr: proj_v_pair(
                vts[c], c, p, tags=("pj", "po"), ptag="po"))
        for dcs in ([0, 1, 2, 3], [4, 5, 6, 7]):
            fillB.append(lambda r=tuple(dcs), h=holder1:
                         proj_qk_piece(qt1, wq_sb, qwt, 0, 1, 0, r, h))
        ct0 = attn_alloc(0)
        attn_block(0, 0, ct0, fillB)

        # Phase C: block (1,0); fillers: pair-0 normalize + Q1-pair1 pieces.
        fillC = [lambda c=ct0: attn_normalize(0, 0, c)]
        for dcs in ([0, 1], [2, 3], [4, 5], [6, 7]):
            fillC.append(lambda r=tuple(dcs), h=holder1:
                         proj_qk_piece(qt1, wq_sb, qwt, 0, 1, 1, r, h))
        ct1 = attn_alloc(1)
        attn_block(1, 0, ct1, fillC)
        prev_norm = (lambda c=ct1: attn_normalize(1, 0, c))

        for b in range(1, NBLK):
            have_next = b + 1 < NBLK
            pp = [[], []]
            if have_next:
                qt = load_chunk("qt", b + 1)
                holder = [None, None]
                for pair in range(2):
                    for dcs in ([0, 1], [2, 3], [4, 5], [6, 7]):
                        pp[pair].append(lambda xt=qt, p=pair, r=tuple(dcs), h=holder:
                                        proj_qk_piece(xt, wq_sb, qwt, 0, b + 1, p, r, h))
            op = [[], []]
            for st in range(4):
                for nh in range(2):
                    op[st // 2].append(lambda s=st, n=nh: out_proj_nh(b - 1, s, n))
            fill0 = [prev_norm] + interleave(pp[0], op[0])
            ct0 = attn_alloc(0)
            attn_block(0, b, ct0, fill0)
            fill1 = [lambda bb=b, c=ct0: attn_normalize(0, bb, c)] + interleave(pp[1], op[1])
            ct1 = attn_alloc(1)
            attn_block(1, b, ct1, fill1)
            prev_norm = (lambda bb=b, c=ct1: attn_normalize(1, bb, c))
        prev_norm()
        # tail: the sc banks are free once the last exp has drained — rotate
        # four PSUM tags so the eight closing out-projections pipeline.
        ttags = ["po", "pj", "sc0", "sc1"]
        for st in range(4):
            for nh in range(2):
                # ACT is idle after the last exp: split the closing PSUM
                # evictions between ACT and DVE so they drain twice as fast.
                out_proj_nh(NBLK - 1, st, nh, tag=ttags[(st * 2 + nh) % 4],
                            scalar_evict=(st * 2 + nh) % 2 == 1)

    nc.compile()
    return nc


_NC_CACHE = None


def _get_nc():
    global _NC_CACHE
    if _NC_CACHE is None:
        _NC_CACHE = _build_nc()
    return _NC_CACHE


def kernel(Q, K, V, Wq, bq, Wk, bk, Wv, bv, Wo, bo, _trace=False, _trace_kwargs=None):
    nc = _get_nc()
    ones2 = np.zeros((2, P), dtype=BF16NP)
    ones2[0, 0:DV] = 1
    ones2[1, DV:2 * DV] = 1
    ident = np.eye(P, dtype=BF16NP)
    qt_h = [np.ascontiguousarray(np.asarray(Q[b]).T).astype(BF16NP) for b in range(B)]
    kt_h = [np.ascontiguousarray(np.asarray(K[b]).T).astype(BF16NP) for b in range(B)]
    vt_h = [np.ascontiguousarray(np.asarray(V[b]).T).astype(BF16NP) for b in range(B)]

    in_maps = []
    for c in range(N_CORES):
        b, g = c % B, c // B
        hs = list(range(g * HPC, (g + 1) * HPC))
        wq_p = np.concatenate([Wq[h] for h in hs], axis=1)
        wk_p = np.concatenate([Wk[h] for h in hs], axis=1)
        wv_p = np.concatenate([Wv[h] * SCALE for h in hs], axis=1)
        bqk_p = np.stack([
            np.concatenate([bq[hs[0]], bq[hs[1]]]),
            np.concatenate([bq[hs[2]], bq[hs[3]]]),
            np.concatenate([bk[hs[0]], bk[hs[1]]]),
            np.concatenate([bk[hs[2]], bk[hs[3]]]),
            np.concatenate([bv[hs[0]], bv[hs[1]]]) * SCALE,
            np.concatenate([bv[hs[2]], bv[hs[3]]]) * SCALE,
        ], axis=1)
        in_maps.append({
            "qt": qt_h[b], "kt": kt_h[b], "vt": vt_h[b],
            "wq": np.ascontiguousarray(wq_p).astype(BF16NP),
            "wk": np.ascontiguousarray(wk_p).astype(BF16NP),
            "wv": np.ascontiguousarray(wv_p).astype(BF16NP),
            "bqk": np.ascontiguousarray(bqk_p.astype(np.float32)),
            "ident": ident,
            "wo": np.ascontiguousarray(Wo[g * HPC * DV:(g + 1) * HPC * DV]).astype(BF16NP),
            "ones2": ones2,
        })

    kw = {}
    if _trace:
        kw = dict(trace=True, **(_trace_kwargs or {}))
    res = run_bass_kernel_spmd(nc, in_maps, core_ids=list(range(N_CORES)), **kw)

    out = np.zeros((B, S, D), dtype=np.float32)
    for c in range(N_CORES):
        out[c % B] += res.results[c]["out"].astype(np.float32)
    out += bo[None, None, :]
    if _trace:
        return out, res
    return out


# revision 23
# speedup vs baseline: 1.1905x; 1.1905x over previous
"""Multi-head attention (B=2, S=2048, D=1024, H=16, dk=dv=64) on 8 TRN2 cores.

Sharding: core c -> batch b = c % 2, head-group g = c // 2 (heads 4g..4g+3).
Each core computes its 4 heads' attention for one batch plus the partial
output projection; the host sums the 4 partials per batch and adds bo.

Host marshalling: inputs are sliced per batch, transposed to [D, S]
(the PE contracts over the partition dim, so projections need D-major
operands), cast to bf16, and the per-head weights are packed/stacked in
bf16; the reference's softmax/dk/2 scale is folded into Wv and bv.

All matmul operands are bf16 (1 cycle/row, and roughly half the PE power
draw of float32r, which matters: the HW activity monitor duty-cycles the
PE to 4/8 under sustained fp32r load). PSUM accumulation stays fp32.

Engine split: PE does matmuls only; ACT does ONLY the exp evictions (it
is the serial bottleneck of the attention inner loop); DVE does proj
PSUM evictions with fused bias + the softmax normalize; GPSIMD (Pool)
does the out-projection PSUM evictions.

Per-core device pipeline:
  1. QWT/KWT [dk, S] head projections (heads pair-stacked on partitions,
     biases fused into the DVE PSUM->SBUF eviction), VW [S, dv] natural
     (via PE transpose; softmax-denominator ones column DMA'd in).
  2. scoresT[t, s] = KWT.T @ QWT per head, two heads concurrently via
     64x128 PE row tiling; exp fused into the PSUM->SBUF eviction (ACT).
     No max-subtraction (|scores| < 50, exp stays finite in fp32/bf16).
  3. ctxT[dv+1, s] = VW1.T @ exp_scoresT accumulated over t; row dv is the
     softmax denominator. Normalize: one K=2 matmul broadcasts both heads'
     denominator rows to 64 partitions each, DVE reciprocal, DVE multiply.
  4. out[s, D] partial = ctx_allT.T @ Wo_slice, Pool-evicted to bf16,
     DMA'd out (host accumulates partials in fp32).
"""
import os
import sys

sys.path.insert(0, "/opt/trn_rl_repo")
os.environ.setdefault("JAX_PLATFORMS", "axon,cpu")

from contextlib import ExitStack

import ml_dtypes
import numpy as np

import concourse.bacc as bacc
import concourse.tile as tile
from concourse import mybir
from concourse.bass_utils import run_bass_kernel_spmd

FP32 = mybir.dt.float32
BF16 = mybir.dt.bfloat16
BF16NP = ml_dtypes.bfloat16

B, S, D = 2, 2048, 1024
H, DK, DV = 16, 64, 64
N_CORES = 8
HPC = H // (N_CORES // B)  # heads per core = 4
P = 128
SBLK = 512                # s-block (free dim of scores matmuls)
NBLK = S // SBLK          # 4
NTT = S // P              # 16 t-tiles
NDC = D // P              # 8 contraction chunks
NV = HPC * (DV + 1)       # 260
SCALE = 1.0 / (DK * 2.0)  # folded into Wv/bv


def _build_nc():
    nc = bacc.Bacc("TRN2", target_bir_lowering=False, debug=False,
                   num_devices=N_CORES)
    d = {}
    for name, shape, dt_ in [
        ("qt", [D, S], BF16), ("kt", [D, S], BF16), ("vt", [D, S], BF16),
        ("wq", [D, 2 * P], BF16), ("wk", [D, 2 * P], BF16), ("wv", [D, 2 * P], BF16),
        ("bqk", [P, 6], FP32), ("ident", [P, P], BF16),
        ("wo", [HPC * DV, D], BF16), ("ones2", [2, P], BF16),
    ]:
        d[name] = nc.dram_tensor(name, shape, dt_, kind="ExternalInput").ap()
    out_d = nc.dram_tensor("out", [S, D], BF16, kind="ExternalOutput").ap()
    # [D, S] viewed as [p, dc, s] chunks for DMA
    xt_view = {
        n: d[n].rearrange("(dc p) s -> p dc s", p=P)
        for n in ("qt", "kt", "vt")
    }

    with tile.TileContext(nc) as tc, ExitStack() as ctx:
        const = ctx.enter_context(tc.tile_pool(name="const", bufs=1))
        wpool = ctx.enter_context(tc.tile_pool(name="wpool", bufs=1))
        xtp = ctx.enter_context(tc.tile_pool(name="xtp", bufs=2))
        projp = ctx.enter_context(tc.tile_pool(name="projp", bufs=1))
        expp = ctx.enter_context(tc.tile_pool(name="expp", bufs=1))
        ctxp = ctx.enter_context(tc.tile_pool(name="ctxp", bufs=1))
        outp = ctx.enter_context(tc.tile_pool(name="outp", bufs=2))
        smallp = ctx.enter_context(tc.tile_pool(name="smallp", bufs=2))
        psum = ctx.enter_context(tc.tile_pool(name="psum", bufs=1, space="PSUM"))

        # ---- weights + full K upfront (K projection starts the kernel;
        # everything else trickles in behind the K chunk loads). Small
        # constants go first: they are needed early and cost ~no latency.
        wk_sb = wpool.tile([P, NDC, 2 * P], BF16)
        kt_sb = wpool.tile([P, NDC, S], BF16)
        wkv = d["wk"].rearrange("(dc p) m -> p dc m", p=P)
        # first K-proj accumulation group touches wk[dc] + kt0[dc] in dc
        # order: interleave half-sized loads so the PE starts ~5us earlier.
        for h in range(2):
            nc.sync.dma_start(wk_sb[:, h * 4:(h + 1) * 4, :],
                              wkv[:, h * 4:(h + 1) * 4, :])
            nc.sync.dma_start(kt_sb[:, h * 4:(h + 1) * 4, 0:SBLK],
                              xt_view["kt"][:, h * 4:(h + 1) * 4, 0:SBLK])
        bqk = const.tile([P, 6], FP32)
        nc.sync.dma_start(bqk[:], d["bqk"])
        ones2 = const.tile([2, P], BF16)
        nc.sync.dma_start(ones2[:], d["ones2"])
        ident_r = const.tile([P, P], BF16)
        nc.sync.dma_start(ident_r[:], d["ident"])
        for ci in range(1, NBLK):
            nc.sync.dma_start(kt_sb[:, :, ci * SBLK:(ci + 1) * SBLK],
                              xt_view["kt"][:, :, ci * SBLK:(ci + 1) * SBLK])
        wq_sb = wpool.tile([P, NDC, 2 * P], BF16)
        wv_sb = wpool.tile([P, NDC, 2 * P], BF16)
        wo_sb = wpool.tile([P, 2, D], BF16)

        def load_w(sb, name, pat="(dc p) m -> p dc m"):
            nc.sync.dma_start(sb[:], d[name].rearrange(pat, p=P))

        # ---- persistent activation tiles ----
        qwt = [projp.tile([P, S], BF16, tag=f"qwt{p_}", name=f"qwt{p_}") for p_ in range(2)]
        kwt = [projp.tile([P, S], BF16, tag=f"kwt{p_}", name=f"kwt{p_}") for p_ in range(2)]
        vwt = [projp.tile([P, S], BF16, tag=f"vwt{p_}", name=f"vwt{p_}") for p_ in range(2)]
        vw = projp.tile([P, NTT, NV], BF16, tag="vw")
        ctx_t = [ctxp.tile([P, S], BF16, tag=f"ctx{p_}", name=f"ctx{p_}") for p_ in range(2)]

        def load_chunk(name, ci, tag="xtk", bufs=2):
            xt = xtp.tile([P, NDC, SBLK], BF16, tag=tag, name="xt", bufs=bufs)
            nc.sync.dma_start(xt[:], xt_view[name][:, :, ci * SBLK:(ci + 1) * SBLK])
            return xt

        def proj_qk_pair(xt, w_sb, dst, bias_col, ci, pair, tag=None, xsl=None):
            """Project one head-pair of a chunk into dst[pair][:, ci*SBLK:...]."""
            pq = psum.tile([P, SBLK], FP32, tag=tag or ("pj" if pair == 0 else "po"),
                           name="pq")
            for dc in range(NDC):
                rhs = xt[:, dc, :] if xsl is None else xt[:, dc, xsl]
                nc.tensor.matmul(pq[:], lhsT=w_sb[:, dc, pair * P:(pair + 1) * P],
                                 rhs=rhs, start=(dc == 0), stop=(dc == NDC - 1))
            nc.vector.tensor_scalar_add(dst[pair][:, ci * SBLK:(ci + 1) * SBLK], pq[:],
                                        bqk[:, bias_col + pair:bias_col + pair + 1])

        def proj_v_pair(xt, ci, pair, tags=("ct0", "ct1"), ptag=None):
            """VWT head-pair (pair-stacked, like Q/K), then PE-transpose into
            vw natural. `tags` picks the transpose PSUM tags (ct0/ct1 are
            free pre-attention; pj/po when run as an attention filler)."""
            proj_qk_pair(xt, wv_sb, vwt, 4, ci, pair, tag=ptag)
            for c in range(SBLK // P):
                tt = ci * (SBLK // P) + c
                tp = psum.tile([P, P], BF16, tag=tags[c % 2], name="tp")
                nc.tensor.transpose(
                    tp[:], vwt[pair][:, ci * SBLK + c * P:ci * SBLK + (c + 1) * P],
                    ident_r[:])
                nc.vector.tensor_copy(
                    vw[:, tt, :].rearrange("p (h v) -> p h v", v=DV + 1)[:, 2 * pair:2 * pair + 2, 0:DV],
                    tp[:].rearrange("p (h v) -> p h v", h=2))

        def attn_alloc(pair):
            return [psum.tile([DV + 1, SBLK], FP32, tag=f"ct{hp}", name=f"ct{hp}")
                    for hp in range(2)]

        def attn_block(pair, b, ct, fillers):
            """Per-2-t-tile pipeline: scores(k) -> exp(k) -> ctx(k), ctx chasing
            exp by one step. One 4-bank scores PSUM per step holds both heads'
            2 t-tiles, evicted by a single FD=2048 exp. `fillers` is a list of
            no-arg callables emitting extra PE work, drained one per step."""
            NK = NTT // 2
            exs = {}
            for k in range(NK + 2):
                if k < NK:
                    sc = [psum.tile([P, 2 * SBLK], FP32, tag=f"sc{hp}", name=f"sc{hp}")
                          for hp in range(2)]
                    for sub in range(2):
                        tt = k * 2 + sub
                        for hp in range(2):
                            lo, hi = hp * DK, (hp + 1) * DK
                            nc.tensor.matmul(
                                sc[hp][:, sub * SBLK:(sub + 1) * SBLK],
                                lhsT=kwt[pair][lo:hi, tt * P:(tt + 1) * P],
                                rhs=qwt[pair][lo:hi, b * SBLK:(b + 1) * SBLK],
                                start=True, stop=True)
                    ex = [expp.tile([P, 2, SBLK], BF16, tag=f"exp{hp}", name=f"exp{hp}", bufs=3)
                          for hp in range(2)]
                    for hp in range(2):
                        nc.scalar.activation(
                            ex[hp][:], sc[hp][:].rearrange("p (u q) -> p u q", u=2),
                            mybir.ActivationFunctionType.Exp)
                    exs[k] = ex
                if fillers:
                    fillers.pop(0)()
                # ctx trails exp by 2 steps: the block's first ctx matmul waits
                # for the ct-psum slot freed by the PREVIOUS block's normalize,
                # so give that chain two steps of slack.
                kc = k - 2
                if kc >= 0:
                    ex = exs.pop(kc)
                    for sub in range(2):
                        tt = kc * 2 + sub
                        for hp in range(2):
                            hh = 2 * pair + hp
                            nc.tensor.matmul(
                                ct[hp][:], lhsT=vw[:, tt, hh * (DV + 1):(hh + 1) * (DV + 1)],
                                rhs=ex[hp][:, sub, :],
                                start=(tt == 0), stop=(tt == NTT - 1))

        def attn_normalize(pair, b, ct):
            # ctx = ct[0:64] * (1 / ct[64]) row-broadcast
            for hp in range(2):
                den = smallp.tile([1, SBLK], BF16, tag="den")
                nc.vector.tensor_copy(den[:], ct[hp][DV:DV + 1, :])
                rb = psum.tile([DV, SBLK], FP32, tag="pj", name="rb")
                nc.tensor.matmul(rb[:], lhsT=ones2[0:1, 0:DV], rhs=den[:],
                                 start=True, stop=True)
                rcp = smallp.tile([DV, SBLK], FP32, tag="rcp")
                nc.vector.reciprocal_approx_fast(rcp[:], rb[:])
                nc.vector.tensor_mul(
                    ctx_t[pair][hp * DV:(hp + 1) * DV, b * SBLK:(b + 1) * SBLK],
                    ct[hp][0:DV, :], rcp[:])

        def out_proj_nh(b, st, nh, tag="po", scalar_evict=False):
            off = b * SBLK + st * P
            po = psum.tile([P, SBLK], FP32, tag=tag, name="po")
            for jc in range(2):
                nc.tensor.matmul(po[:],
                                 lhsT=ctx_t[jc][:, off:off + P],
                                 rhs=wo_sb[:, jc, nh * SBLK:(nh + 1) * SBLK],
                                 start=(jc == 0), stop=(jc == 1))
            ob = outp.tile([P, SBLK], BF16, tag="ob", bufs=4)
            if scalar_evict:
                nc.scalar.copy(ob[:], po[:])
            else:
                nc.vector.tensor_copy(ob[:], po[:])
            nc.sync.dma_start(out_d[off:off + P, nh * SBLK:(nh + 1) * SBLK], ob[:])

        def proj_qk_piece(xt, w_sb, dst, bias_col, ci, pair, dc_range, pq_holder):
            if dc_range[0] == 0:
                pq_holder[pair] = psum.tile([P, SBLK], FP32, tag="pj", name="pq")
            pq = pq_holder[pair]
            for dc in dc_range:
                nc.tensor.matmul(pq[:], lhsT=w_sb[:, dc, pair * P:(pair + 1) * P],
                                 rhs=xt[:, dc, :], start=(dc == 0), stop=(dc == NDC - 1))
            if dc_range[-1] == NDC - 1:
                nc.vector.tensor_scalar_add(dst[pair][:, ci * SBLK:(ci + 1) * SBLK], pq[:],
                                            bqk[:, bias_col + pair:bias_col + pair + 1])

        # ---- emission schedule (pair-split early attention) ----
        # Pair-0's K/Q0/V projections run first; attention block (0,0)
        # starts immediately after, with pair-1's K/V/Q0 projections drained
        # into its per-step PE slack. This puts the first exp on ACT (the
        # serial bottleneck) ~25us earlier than projecting everything first.
        qt0 = load_chunk("qt", 0)
        load_w(wq_sb, "wq")
        load_w(wv_sb, "wv")
        vts = {ci: load_chunk("vt", ci, tag="xtv", bufs=4) for ci in range(NBLK)}
        # softmax-denominator ones column (strided over the 65-wide head
        # slots): engine memset, not DMA — an engine op gets proper ordering
        # against the ctx matmuls that read it, and doesn't queue behind
        # ~10MB of input DMA.
        for hh in range(HPC):
            nc.vector.memset(vw[:, :, hh * (DV + 1) + DV], 1.0)
        load_w(wo_sb, "wo", "(jc p) n -> p jc n")

        # Phase A: K (both pairs, alternating PSUM tags) and Q0 only. All V
        # projections drain into block (0,0)'s filler slots below: ctx step
        # k only needs V chunk k//2, so pair-0 chunk ci is due at step
        # 2ci+2, and pair-1 slots aren't read until block (1,0).
        for ci in range(NBLK):
            for pair in range(2):
                proj_qk_pair(kt_sb, wk_sb, kwt, 2, ci, pair,
                             xsl=slice(ci * SBLK, (ci + 1) * SBLK))
        proj_qk_pair(qt0, wq_sb, qwt, 0, 0, 0)
        proj_qk_pair(qt0, wq_sb, qwt, 0, 0, 1)

        def interleave(a, bl):
            out = []
            for i in range(max(len(a), len(bl))):
                if i < len(a):
                    out.append(a[i])
                if i < len(bl):
                    out.append(bl[i])
            return out

        # Phase B: block (0,0); fillers: all V projections (deadline order).
        qt1 = load_chunk("qt", 1)
        holder1 = [None, None]
        fillB = []
        for ci, pr in [(0, 0), (1, 0), (0, 1), (2, 0), (1, 1), (3, 0),
                       (2, 1), (3, 1)]:
            fillB.append(lambda c=ci, p=pr: proj_v_pair(
                vts[c], c, p, tags=("pj", "po"), ptag="po"))
        for dcs in ([0, 1, 2, 3], [4, 5, 6, 7]):
            fillB.append(lambda r=tuple(dcs), h=holder1:
                         proj_qk_piece(qt1, wq_sb, qwt, 0, 1, 0, r, h))
        ct0 = attn_alloc(0)
        attn_block(0, 0, ct0, fillB)

        # Phase C: block (1,0); fillers: pair-0 normalize + Q1-pair1 pieces.
        fillC = [lambda c=ct0: attn_normalize(0, 0, c)]
        for dcs in ([0, 1], [2, 3], [4, 5], [6, 7]):
            fillC.append(lambda r=tuple(dcs), h=holder1:
                         proj_qk_piece(qt1, wq_sb, qwt, 0, 1, 1, r, h))
        ct1 = attn_alloc(1)
        attn_block(1, 0, ct1, fillC)
        prev_norm = (lambda c=ct1: attn_normalize(1, 0, c))

        for b in range(1, NBLK):
            have_next = b + 1 < NBLK
            pp = [[], []]
            if have_next:
                qt = load_chunk("qt", b + 1)
                holder = [None, None]
                for pair in range(2):
                    for dcs in ([0, 1], [2, 3], [4, 5], [6, 7]):
                        pp[pair].append(lambda xt=qt, p=pair, r=tuple(dcs), h=holder:
                                        proj_qk_piece(xt, wq_sb, qwt, 0, b + 1, p, r, h))
            op = [[], []]
            for st in range(4):
                for nh in range(2):
                    op[st // 2].append(lambda s=st, n=nh: out_proj_nh(b - 1, s, n))
            fill0 = [prev_norm] + interleave(pp[0], op[0])
            ct0 = attn_alloc(0)
            attn_block(0, b, ct0, fill0)
            fill1 = [lambda bb=b, c=ct0: attn_normalize(0, bb, c)] + interleave(pp[1], op[1])
            ct1 = attn_alloc(1)
            attn_block(1, b, ct1, fill1)
            prev_norm = (lambda bb=b, c=ct1: attn_normalize(1, bb, c))
        prev_norm()
        # tail: the sc banks are free once the last exp has drained — rotate
        # four PSUM tags so the eight closing out-projections pipeline.
        ttags = ["po", "pj", "sc0", "sc1"]
        for st in range(4):
            for nh in range(2):
                # ACT is idle after the last exp: split the closing PSUM
                # evictions between ACT and DVE so they drain twice as fast.
                out_proj_nh(NBLK - 1, st, nh, tag=ttags[(st * 2 + nh) % 4],
                            scalar_evict=(st * 2 + nh) % 2 == 1)

    nc.compile()
    return nc


_NC_CACHE = None


def _get_nc():
    global _NC_CACHE
    if _NC_CACHE is None:
        _NC_CACHE = _build_nc()
    return _NC_CACHE


def kernel(Q, K, V, Wq, bq, Wk, bk, Wv, bv, Wo, bo, _trace=False, _trace_kwargs=None):
    nc = _get_nc()
    ones2 = np.zeros((2, P), dtype=BF16NP)
    ones2[0, 0:DV] = 1
    ones2[1, DV:2 * DV] = 1
    ident = np.eye(P, dtype=BF16NP)
    qt_h = [np.ascontiguousarray(np.asarray(Q[b]).T).astype(BF16NP) for b in range(B)]
    kt_h = [np.ascontiguousarray(np.asarray(K[b]).T).astype(BF16NP) for b in range(B)]
    vt_h = [np.ascontiguousarray(np.asarray(V[b]).T).astype(BF16NP) for b in range(B)]

    in_maps = []
    for c in range(N_CORES):
        b, g = c % B, c // B
        hs = list(range(g * HPC, (g + 1) * HPC))
        wq_p = np.concatenate([Wq[h] for h in hs], axis=1)
        wk_p = np.concatenate([Wk[h] for h in hs], axis=1)
        wv_p = np.concatenate([Wv[h] * SCALE for h in hs], axis=1)
        bqk_p = np.stack([
            np.concatenate([bq[hs[0]], bq[hs[1]]]),
            np.concatenate([bq[hs[2]], bq[hs[3]]]),
            np.concatenate([bk[hs[0]], bk[hs[1]]]),
            np.concatenate([bk[hs[2]], bk[hs[3]]]),
            np.concatenate([bv[hs[0]], bv[hs[1]]]) * SCALE,
            np.concatenate([bv[hs[2]], bv[hs[3]]]) * SCALE,
        ], axis=1)
        in_maps.append({
            "qt": qt_h[b], "kt": kt_h[b], "vt": vt_h[b],
            "wq": np.ascontiguousarray(wq_p).astype(BF16NP),
            "wk": np.ascontiguousarray(wk_p).astype(BF16NP),
            "wv": np.ascontiguousarray(wv_p).astype(BF16NP),
            "bqk": np.ascontiguousarray(bqk_p.astype(np.float32)),
            "ident": ident,
            "wo": np.ascontiguousarray(Wo[g * HPC * DV:(g + 1) * HPC * DV]).astype(BF16NP),
            "ones2": ones2,
        })

    kw = {}
    if _trace:
        kw = dict(trace=True, **(_trace_kwargs or {}))
    res = run_bass_kernel_spmd(nc, in_maps, core_ids=list(range(N_CORES)), **kw)

    out = np.zeros((B, S, D), dtype=np.float32)
    for c in range(N_CORES):
        out[c % B] += res.results[c]["out"].astype(np.float32)
    out += bo[None, None, :]
    if _trace:
        return out, res
    return out
